# revision 30
# baseline (speedup 1.0000x reference)
"""Trainium2 Bass kernel for the DGM kNN problem.

Computation (per batch b):
  xe = x @ W                                   [4096, 256]
  D_ij = relu(|xe_i|^2 + |xe_j|^2 - 2 xe_i.xe_j)
  score_ij = ln(-ln(q_ij)) - exp(clip(T,-5,5)) * D_ij     (= -lq)
  top-16 largest score per row i -> (logprobs, indices) -> edges

Sharding: 8 cores = 4 batches x 2 row-halves. Each core receives its
batch's x ROTATED so its own 2048 rows come first (the program is
identical on every core; only data differs). Each core computes the
full 4096x256 embedding (needed for the key side), distances of its
2048 rows against all 4096 keys, and a segmented top-8 per 1024-key
segment (32 candidates/row: values + in-segment indices). The host
merges the 32 candidates into the exact top-16 (jax top_k tie
semantics), with an exact-detected, rare fallback for rows where one
segment hides >8 of the true top-16.

All arithmetic is fp32 and follows the reference's association order
where it matters (relu/scale/subtract orderings are bit-matching; the
dot/sq reductions differ only by summation order, which is inherent to
any reimplementation).
"""

import os
import numpy as np

B, N, D, K = 4, 4096, 256, 16
RPC = N // 2              # rows per core
P = 128                   # partitions
NSEG, SEG = 8, 512        # top-k segments per row
NCH = N // 512            # 512-wide psum chunks per row-block
NBLK_ALL = N // P         # 32 key blocks
NBLK_OWN = RPC // P       # 16 row blocks owned per core

_CACHE = {}


def _build_program():
    import concourse.bacc as bacc
    import concourse.mybir as mybir
    import concourse.tile as tile
    from concourse.masks import make_identity

    f32 = mybir.dt.float32
    u16 = mybir.dt.uint16
    AF = mybir.ActivationFunctionType
    ADD = mybir.AluOpType.add

    nc = bacc.Bacc("TRN2", target_bir_lowering=False, debug=False)

    xb_d = nc.dram_tensor("xb", [N, D], f32, kind="ExternalInput")
    w_d = nc.dram_tensor("w", [D, D], f32, kind="ExternalInput")
    qr_d = nc.dram_tensor("qr", [RPC, N], f32, kind="ExternalInput")
    t_d = nc.dram_tensor("temp", [1], f32, kind="ExternalInput")

    xe_d = nc.dram_tensor("xe_out", [RPC, D], f32, kind="ExternalOutput")
    cv_d = nc.dram_tensor("cand_v", [RPC, NSEG * 8], f32, kind="ExternalOutput")
    ci_d = nc.dram_tensor("cand_i", [RPC, NSEG * 8], u16, kind="ExternalOutput")
    sq_scratch_d = nc.dram_tensor("sq_scratch", [NBLK_ALL, P], f32)

    xb_t = xb_d.ap().rearrange("(t p) d -> t p d", p=P)       # [32,128,256]
    w_t = w_d.ap().rearrange("(b p) e -> b p e", p=P)         # [2,128,256]
    qr_t = qr_d.ap().rearrange("(t p) n -> t p n", p=P)       # [16,128,4096]
    xe_t = xe_d.ap().rearrange("(t p) d -> t p d", p=P)       # [16,128,256]
    cv_t = cv_d.ap().rearrange("(t p) c -> t p c", p=P)
    ci_t = ci_d.ap().rearrange("(t p) c -> t p c", p=P)

    with tile.TileContext(nc) as tc:
        with tc.tile_pool(name="const", bufs=1) as const_pool:
            w_sb = const_pool.tile([P, 2, D], f32)
            xeT = const_pool.tile([P, 2, N], f32)
            xeTm2 = const_pool.tile([P, 2, N], f32)  # 2t * xeT (dist lhsT)
            bcast = const_pool.tile([P, N], f32)     # -t * sq_j broadcast
            sq_all = const_pool.tile([P, NBLK_ALL], f32)
            msqi = const_pool.tile([P, NBLK_OWN], f32)  # -t * sq_i (rows)
            t_bc = const_pool.tile([P, 1], f32)
            t2_bc = const_pool.tile([P, 1], f32)
            mt_bc = const_pool.tile([P, 1], f32)
            ident = const_pool.tile([P, P], f32)
            sq_flat = const_pool.tile([1, N], f32)

            make_identity(nc, ident)
            for db in range(2):
                nc.sync.dma_start(w_sb[:, db, :], w_t[db])

            # temperature -> t_bc = exp(clip(T, -5, 5)) on all partitions,
            # plus 2t and -t variants (exact *2 / *-1 scalings)
            tt = const_pool.tile([1, 1], f32)
            nc.sync.dma_start(tt, t_d.ap().rearrange("(o a) -> o a", o=1))
            nc.vector.tensor_scalar_max(tt, tt, -5.0)
            nc.vector.tensor_scalar_min(tt, tt, 5.0)
            nc.scalar.activation(tt, tt, AF.Exp)
            nc.gpsimd.partition_broadcast(t_bc, tt)
            nc.vector.tensor_scalar_mul(t2_bc, t_bc, 2.0)
            nc.vector.tensor_scalar_mul(mt_bc, t_bc, -1.0)

            # ---------------- setup: xT, xeT(+m2), xe, sq, bcast ----------
            with (
                tc.tile_pool(name="su_in", bufs=6) as su_in,
                tc.tile_pool(name="su_sb", bufs=4) as su_sb,
                tc.tile_pool(name="su_xt", bufs=1) as su_xt,
                tc.tile_pool(name="su_ps", bufs=4, space="PSUM") as su_ps,
                tc.tile_pool(name="su_ps2", bufs=2, space="PSUM") as su_ps2,
                tc.tile_pool(name="su_ps3", bufs=2, space="PSUM") as su_ps3,
            ):
                xT = su_xt.tile([P, 2, N], f32)
                # x -> xT via PE transposes of [128,128] blocks
                for t in range(NBLK_ALL):
                    xin = su_in.tile([P, D], f32, tag="xin")
                    nc.sync.dma_start(xin, xb_t[t])
                    for db in range(2):
                        pst = su_ps.tile([P, P], f32, tag="pst")
                        nc.tensor.transpose(pst, xin[:, db * P:(db + 1) * P], ident)
                        nc.vector.tensor_copy(xT[:, db, t * P:(t + 1) * P], pst)

                # per 512-key chunk: xeT (and -2*xeT) via matmul, then xe
                # row-blocks + sq for the 4 key-blocks covered; every 1024
                # keys completed feeds one bcast chunk of sq_j.
                for ch in range(NCH):
                    for eb in range(2):
                        ps = su_ps2.tile([P, 512], f32, tag="mm")
                        for db in range(2):
                            nc.tensor.matmul(
                                ps,
                                w_sb[:, db, eb * P:(eb + 1) * P],
                                xT[:, db, ch * 512:(ch + 1) * 512],
                                start=(db == 0),
                                stop=(db == 1),
                            )
                        nc.scalar.copy(xeT[:, eb, ch * 512:(ch + 1) * 512], ps)
                        nc.scalar.activation(
                            xeTm2[:, eb, ch * 512:(ch + 1) * 512], ps,
                            AF.Identity, scale=t2_bc)
                    for t in range(4 * ch, 4 * ch + 4):
                        xesb = su_sb.tile([P, D], f32, tag="xesb")
                        for eb in range(2):
                            pst = su_ps.tile([P, P], f32, tag="pst")
                            nc.tensor.transpose(
                                pst, xeT[:, eb, t * P:(t + 1) * P], ident)
                            nc.vector.tensor_copy(
                                xesb[:, eb * P:(eb + 1) * P], pst)
                        scr = su_sb.tile([P, D], f32, tag="sqscr")
                        nc.scalar.activation(scr, xesb, AF.Square,
                                             accum_out=sq_all[:, t:t + 1])
                        if t < NBLK_OWN:
                            nc.sync.dma_start(xe_t[t], xesb)
                    if ch % 2 == 1:
                        # keys [c*1024, (c+1)*1024) complete -> bcast chunk c
                        c = ch // 2
                        pssq = su_ps3.tile([8, P], f32, tag="sqt")
                        nc.tensor.transpose(
                            pssq, sq_all[:, c * 8:(c + 1) * 8], ident)
                        sqT = su_sb.tile([8, P], f32, tag="sqT")
                        nc.vector.tensor_copy(sqT, pssq)
                        nc.sync.dma_start(
                            sq_scratch_d.ap()[c * 8:(c + 1) * 8, :], sqT)
                        nc.sync.dma_start(
                            sq_flat[:, c * 1024:(c + 1) * 1024],
                            sq_scratch_d.ap()[c * 8:(c + 1) * 8, :]
                            .rearrange("a b -> (a b)")
                            .rearrange("(o n) -> o n", o=1))
                        # scale to -t*sq_j before broadcasting
                        nc.scalar.activation(
                            sq_flat[:, c * 1024:(c + 1) * 1024],
                            sq_flat[:, c * 1024:(c + 1) * 1024],
                            AF.Identity, scale=mt_bc[0:1, :])
                        nc.gpsimd.partition_broadcast(
                            bcast[:, c * 1024:(c + 1) * 1024],
                            sq_flat[:, c * 1024:(c + 1) * 1024])
                        if c < 2:
                            # -t*sq_i for row blocks 8c..8c+7 (our rows)
                            nc.scalar.activation(
                                msqi[:, c * 8:(c + 1) * 8],
                                sq_all[:, c * 8:(c + 1) * 8],
                                AF.Identity, scale=mt_bc)

            # ---------------- main loop over our 16 row blocks ------------
            with (
                tc.tile_pool(name="qp", bufs=3) as qp,
                tc.tile_pool(name="sp", bufs=3) as sp,
                tc.tile_pool(name="cvp", bufs=2) as cvp,
                tc.tile_pool(name="cip", bufs=2) as cip,
                tc.tile_pool(name="dps", bufs=4, space="PSUM") as dps,
            ):
                for r in range(NBLK_OWN):
                    q_tile = qp.tile([P, N], f32, tag="q")
                    nc.sync.dma_start(q_tile, qr_t[r])
                    # g2 = ln(-ln(q)) in place
                    nc.scalar.activation(q_tile, q_tile, AF.Ln)
                    nc.scalar.activation(q_tile, q_tile, AF.Ln, scale=-1.0)

                    s_tile = sp.tile([P, N], f32, tag="s")
                    for j2 in range(NCH // 2):
                        ps = dps.tile([P, 2, 512], f32, tag="dot")
                        for jj in range(2):
                            j = j2 * 2 + jj
                            for db in range(2):
                                nc.tensor.matmul(
                                    ps[:, jj, :],
                                    xeTm2[:, db, r * P:(r + 1) * P],
                                    xeT[:, db, j * 512:(j + 1) * 512],
                                    start=(db == 0),
                                    stop=(db == 1),
                                )
                        # psum holds 2t*dot; u = -t*E = (2t*dot - t*sq_i)
                        # - t*sq_j.  Drains split between DVE and ACT+gpsimd
                        # so neither in-order stream starves the PE of psum.
                        sl = slice(j2 * 1024, (j2 + 1) * 1024)
                        if j2 < 2:
                            nc.vector.scalar_tensor_tensor(
                                s_tile[:, sl], ps.rearrange("p a b -> p (a b)"),
                                msqi[:, r:r + 1], bcast[:, sl],
                                op0=ADD, op1=ADD)
                        else:
                            nc.scalar.activation(
                                s_tile[:, sl], ps.rearrange("p a b -> p (a b)"),
                                AF.Identity, bias=msqi[:, r:r + 1], scale=1.0)
                            nc.gpsimd.tensor_add(
                                s_tile[:, sl], s_tile[:, sl], bcast[:, sl])
                    # score = g2 - t*relu(E) == g2 + min(u, 0), one fused op
                    # per half-row (split across DVE/gpsimd for balance)
                    MIN = mybir.AluOpType.min
                    h0 = slice(0, N // 2)
                    h1 = slice(N // 2, N)
                    nc.vector.scalar_tensor_tensor(
                        q_tile[:, h0], s_tile[:, h0], 0.0, q_tile[:, h0],
                        op0=MIN, op1=ADD)
                    if r < NBLK_OWN - 1:
                        nc.gpsimd.tensor_scalar_min(
                            s_tile[:, h1], s_tile[:, h1], 0.0)
                        nc.gpsimd.tensor_add(
                            q_tile[:, h1], q_tile[:, h1], s_tile[:, h1])
                    else:
                        # final block: keep the tail chain on the faster DVE
                        nc.vector.scalar_tensor_tensor(
                            q_tile[:, h1], s_tile[:, h1], 0.0, q_tile[:, h1],
                            op0=MIN, op1=ADD)

                    cv = cvp.tile([P, NSEG * 8], f32, tag="cv")
                    ci = cip.tile([P, NSEG * 8], u16, tag="ci")
                    for s in range(NSEG):
                        nc.vector.max(out=cv[:, s * 8:(s + 1) * 8],
                                      in_=q_tile[:, s * SEG:(s + 1) * SEG])
                    for s in range(NSEG):
                        nc.vector.max_index(ci[:, s * 8:(s + 1) * 8],
                                            cv[:, s * 8:(s + 1) * 8],
                                            q_tile[:, s * SEG:(s + 1) * SEG])
                    nc.sync.dma_start(cv_t[r], cv)
                    nc.sync.dma_start(ci_t[r], ci)

    nc.compile()
    return nc


def _get_program():
    if "nc" not in _CACHE:
        _CACHE["nc"] = _build_program()
    return _CACHE["nc"]


def _run_on_hw(in_maps, trace=False):
    from concourse.bass_utils import run_bass_kernel_spmd
    nc = _get_program()
    if trace:
        try:
            return run_bass_kernel_spmd(
                nc, in_maps, core_ids=list(range(8)), trace=True)
        except Exception as e:  # profiling infra unavailable -> plain run
            print(f"trace run failed ({type(e).__name__}: {e}); retrying untraced")
    return run_bass_kernel_spmd(nc, in_maps, core_ids=list(range(8)), trace=False)


def _host_topk(V, GI):
    """Exact top-16 (desc value, ties -> lower global index) per row."""
    order = np.lexsort((GI, -V.astype(np.float64)), axis=-1)[:, :K]
    val16 = np.take_along_axis(V, order, 1)
    idx16 = np.take_along_axis(GI, order, 1)
    return val16, idx16


def kernel(x, A, W, temperature, q):
    x = np.asarray(x, dtype=np.float32)
    W = np.asarray(W, dtype=np.float32)
    q = np.asarray(q, dtype=np.float32)
    temperature = np.asarray(temperature, dtype=np.float32)

    in_maps = []
    for c in range(8):
        b, h = c // 2, c % 2
        off = h * RPC
        xb = np.roll(x[b], -off, axis=0) if h else x[b]
        qr = q[b, off:off + RPC]
        if h:
            qr = np.roll(qr, -off, axis=1)
        in_maps.append({
            "xb": np.ascontiguousarray(xb),
            "w": W,
            "qr": np.ascontiguousarray(qr),
            "temp": temperature,
        })

    trace = bool(int(os.environ.get("DGM_TRACE", "0")))
    kr = _run_on_hw(in_maps, trace=trace)
    results = kr.results
    if trace and kr.exec_time_ns is not None:
        _CACHE["exec_time_ns"] = kr.exec_time_ns

    # ---- assemble xe ----
    xe = np.empty((B, N, D), dtype=np.float32)
    for c in range(8):
        b, h = c // 2, c % 2
        xe[b, h * RPC:(h + 1) * RPC] = results[c]["xe_out"]

    # ---- candidates -> exact top-16 per row ----
    seg_off = np.repeat(np.arange(NSEG, dtype=np.int32) * SEG, 8)
    t_sc = np.float32(np.exp(np.float32(np.clip(temperature[0], -5.0, 5.0))))

    logprobs = np.empty((B, N, K), dtype=np.float32)
    indices = np.empty((B, N, K), dtype=np.int32)
    n_fallback = 0
    for b in range(B):
        V = np.concatenate(
            [results[2 * b]["cand_v"], results[2 * b + 1]["cand_v"]], axis=0)
        L = np.concatenate(
            [results[2 * b]["cand_i"], results[2 * b + 1]["cand_i"]],
            axis=0).astype(np.int32)
        GI = L + seg_off[None, :]
        GI[RPC:] = (GI[RPC:] + RPC) % N  # undo the key rotation of half 1
        val16, idx16 = _host_topk(V, GI)

        # A segment can hide a relevant element (a 9th element >= the 16th
        # winner, or tied with it) iff its 8th candidate value >= the 16th
        # winner.  Exact condition.
        seg8 = V.reshape(N, NSEG, 8)[:, :, 7]
        bad = (seg8 >= val16[:, K - 1:K]).any(axis=1)
        rows = np.nonzero(bad)[0]
        n_fallback += len(rows)
        if len(rows):
            xeb = xe[b]
            sqb = (xeb * xeb).sum(axis=1, dtype=np.float32)
            dots = (xeb @ xeb[rows].T).T.astype(np.float32)  # [nbad, N]
            E = sqb[rows][:, None] + sqb[None, :] - np.float32(2.0) * dots
            lq = np.maximum(E, np.float32(0.0)) * t_sc \
                - np.log(-np.log(q[b, rows])).astype(np.float32)
            orderr = np.lexsort(
                (np.broadcast_to(np.arange(N), lq.shape), lq), axis=-1)[:, :K]
            idx16[rows] = orderr.astype(np.int32)
            val16[rows] = -np.take_along_axis(lq, orderr, 1)
        logprobs[b] = val16
        indices[b] = idx16
    _CACHE["n_fallback"] = n_fallback

    # ---- edges (reference arithmetic, int32) ----
    rows_arr = np.broadcast_to(
        np.arange(N, dtype=np.int32)[None, :, None], (B, N, K))
    edges = np.stack(
        (indices.reshape(B, -1), rows_arr.reshape(B, -1)), axis=-2)
    offset = (np.arange(B, dtype=np.int32) * N)[:, None, None]
    edges_sparse = np.transpose(edges + offset, (1, 0, 2)).reshape(2, -1)

    return xe, edges_sparse, logprobs


# revision 31
# speedup vs baseline: 1.9941x; 1.9941x over previous
"""Trainium2 Bass kernel for the DGM kNN problem.

Computation (per batch b):
  xe = x @ W                                   [4096, 256]
  D_ij = relu(|xe_i|^2 + |xe_j|^2 - 2 xe_i.xe_j)
  score_ij = ln(-ln(q_ij)) - exp(clip(T,-5,5)) * D_ij     (= -lq)
  top-16 largest score per row i -> (logprobs, indices) -> edges

Sharding: 8 cores = 4 batches x 2 row-halves. Each core receives its
batch's x ROTATED so its own 2048 rows come first (the program is
identical on every core; only data differs). Each core computes the
full 4096x256 embedding (needed for the key side), distances of its
2048 rows against all 4096 keys, and a segmented top-8 per 1024-key
segment (32 candidates/row: values + in-segment indices). The host
merges the 32 candidates into the exact top-16 (jax top_k tie
semantics), with an exact-detected, rare fallback for rows where one
segment hides >8 of the true top-16.

All arithmetic is fp32 and follows the reference's association order
where it matters (relu/scale/subtract orderings are bit-matching; the
dot/sq reductions differ only by summation order, which is inherent to
any reimplementation).
"""

import os
import numpy as np

B, N, D, K = 4, 4096, 256, 16
RPC = N // 2              # rows per core
P = 128                   # partitions
NSEG, SEG = 8, 512        # top-k segments per row
NCH = N // 512            # 512-wide psum chunks per row-block
NBLK_ALL = N // P         # 32 key blocks
NBLK_OWN = RPC // P       # 16 row blocks owned per core

_CACHE = {}


def _build_program():
    import concourse.bacc as bacc
    import concourse.mybir as mybir
    import concourse.tile as tile
    from concourse.masks import make_identity

    f32 = mybir.dt.float32
    u16 = mybir.dt.uint16
    AF = mybir.ActivationFunctionType
    ADD = mybir.AluOpType.add

    nc = bacc.Bacc("TRN2", target_bir_lowering=False, debug=False)

    xb_d = nc.dram_tensor("xb", [N, D], f32, kind="ExternalInput")
    w_d = nc.dram_tensor("w", [D, D], f32, kind="ExternalInput")
    qr_d = nc.dram_tensor("qr", [RPC, N], f32, kind="ExternalInput")
    t_d = nc.dram_tensor("temp", [1], f32, kind="ExternalInput")

    xe_d = nc.dram_tensor("xe_out", [RPC, D], f32, kind="ExternalOutput")
    cv_d = nc.dram_tensor("cand_v", [RPC, NSEG * 8], f32, kind="ExternalOutput")
    ci_d = nc.dram_tensor("cand_i", [RPC, NSEG * 8], u16, kind="ExternalOutput")
    sq_scratch_d = nc.dram_tensor("sq_scratch", [NBLK_ALL, P], f32)

    xb_t = xb_d.ap().rearrange("(t p) d -> t p d", p=P)       # [32,128,256]
    w_t = w_d.ap().rearrange("(b p) e -> b p e", p=P)         # [2,128,256]
    qr_t = qr_d.ap().rearrange("(t p) n -> t p n", p=P)       # [16,128,4096]
    xe_t = xe_d.ap().rearrange("(t p) d -> t p d", p=P)       # [16,128,256]
    cv_t = cv_d.ap().rearrange("(t p) c -> t p c", p=P)
    ci_t = ci_d.ap().rearrange("(t p) c -> t p c", p=P)

    with tile.TileContext(nc) as tc:
        with tc.tile_pool(name="const", bufs=1) as const_pool:
            w_sb = const_pool.tile([P, 2, D], f32)
            xeT = const_pool.tile([P, 2, N], f32)
            xeTm2 = const_pool.tile([P, 2, N], f32)  # -2 * xeT (dist lhsT)
            bcast = const_pool.tile([P, N], f32)
            sq_all = const_pool.tile([P, NBLK_ALL], f32)
            t_bc = const_pool.tile([P, 1], f32)
            ident = const_pool.tile([P, P], f32)
            sq_flat = const_pool.tile([1, N], f32)

            make_identity(nc, ident)
            for db in range(2):
                nc.sync.dma_start(w_sb[:, db, :], w_t[db])

            # temperature -> t_bc = exp(clip(T, -5, 5)) on all partitions
            tt = const_pool.tile([1, 1], f32)
            nc.sync.dma_start(tt, t_d.ap().rearrange("(o a) -> o a", o=1))
            nc.vector.tensor_scalar_max(tt, tt, -5.0)
            nc.vector.tensor_scalar_min(tt, tt, 5.0)
            nc.scalar.activation(tt, tt, AF.Exp)
            nc.gpsimd.partition_broadcast(t_bc, tt)

            # ---------------- setup: xT, xeT(+m2), xe, sq, bcast ----------
            with (
                tc.tile_pool(name="su_in", bufs=6) as su_in,
                tc.tile_pool(name="su_sb", bufs=4) as su_sb,
                tc.tile_pool(name="su_xt", bufs=1) as su_xt,
                tc.tile_pool(name="su_ps", bufs=4, space="PSUM") as su_ps,
                tc.tile_pool(name="su_ps2", bufs=2, space="PSUM") as su_ps2,
                tc.tile_pool(name="su_ps3", bufs=2, space="PSUM") as su_ps3,
            ):
                xT = su_xt.tile([P, 2, N], f32)
                # x -> xT via PE transposes of [128,128] blocks
                for t in range(NBLK_ALL):
                    xin = su_in.tile([P, D], f32, tag="xin")
                    nc.sync.dma_start(xin, xb_t[t])
                    for db in range(2):
                        pst = su_ps.tile([P, P], f32, tag="pst")
                        nc.tensor.transpose(pst, xin[:, db * P:(db + 1) * P], ident)
                        nc.vector.tensor_copy(xT[:, db, t * P:(t + 1) * P], pst)

                # per 512-key chunk: xeT (and -2*xeT) via matmul, then xe
                # row-blocks + sq for the 4 key-blocks covered; every 1024
                # keys completed feeds one bcast chunk of sq_j.
                for ch in range(NCH):
                    for eb in range(2):
                        ps = su_ps2.tile([P, 512], f32, tag="mm")
                        for db in range(2):
                            nc.tensor.matmul(
                                ps,
                                w_sb[:, db, eb * P:(eb + 1) * P],
                                xT[:, db, ch * 512:(ch + 1) * 512],
                                start=(db == 0),
                                stop=(db == 1),
                            )
                        nc.scalar.copy(xeT[:, eb, ch * 512:(ch + 1) * 512], ps)
                        nc.scalar.activation(
                            xeTm2[:, eb, ch * 512:(ch + 1) * 512], ps,
                            AF.Identity, scale=-2.0)
                    for t in range(4 * ch, 4 * ch + 4):
                        xesb = su_sb.tile([P, D], f32, tag="xesb")
                        for eb in range(2):
                            pst = su_ps.tile([P, P], f32, tag="pst")
                            nc.tensor.transpose(
                                pst, xeT[:, eb, t * P:(t + 1) * P], ident)
                            nc.vector.tensor_copy(
                                xesb[:, eb * P:(eb + 1) * P], pst)
                        scr = su_sb.tile([P, D], f32, tag="sqscr")
                        nc.scalar.activation(scr, xesb, AF.Square,
                                             accum_out=sq_all[:, t:t + 1])
                        if t < NBLK_OWN:
                            nc.sync.dma_start(xe_t[t], xesb)
                    if ch % 2 == 1:
                        # keys [c*1024, (c+1)*1024) complete -> bcast chunk c
                        c = ch // 2
                        pssq = su_ps3.tile([8, P], f32, tag="sqt")
                        nc.tensor.transpose(
                            pssq, sq_all[:, c * 8:(c + 1) * 8], ident)
                        sqT = su_sb.tile([8, P], f32, tag="sqT")
                        nc.vector.tensor_copy(sqT, pssq)
                        nc.sync.dma_start(
                            sq_scratch_d.ap()[c * 8:(c + 1) * 8, :], sqT)
                        nc.sync.dma_start(
                            sq_flat[:, c * 1024:(c + 1) * 1024],
                            sq_scratch_d.ap()[c * 8:(c + 1) * 8, :]
                            .rearrange("a b -> (a b)")
                            .rearrange("(o n) -> o n", o=1))
                        nc.gpsimd.partition_broadcast(
                            bcast[:, c * 1024:(c + 1) * 1024],
                            sq_flat[:, c * 1024:(c + 1) * 1024])

            # ---------------- main loop over our 16 row blocks ------------
            with (
                tc.tile_pool(name="qp", bufs=3) as qp,
                tc.tile_pool(name="sp", bufs=3) as sp,
                tc.tile_pool(name="cvp", bufs=2) as cvp,
                tc.tile_pool(name="cip", bufs=2) as cip,
                tc.tile_pool(name="dps", bufs=4, space="PSUM") as dps,
            ):
                for r in range(NBLK_OWN):
                    q_tile = qp.tile([P, N], f32, tag="q")
                    nc.sync.dma_start(q_tile, qr_t[r])
                    # g2 = ln(-ln(q)) in place
                    nc.scalar.activation(q_tile, q_tile, AF.Ln)
                    nc.scalar.activation(q_tile, q_tile, AF.Ln, scale=-1.0)

                    s_tile = sp.tile([P, N], f32, tag="s")
                    for j2 in range(NCH // 2):
                        ps = dps.tile([P, 2, 512], f32, tag="dot")
                        for jj in range(2):
                            j = j2 * 2 + jj
                            for db in range(2):
                                nc.tensor.matmul(
                                    ps[:, jj, :],
                                    xeTm2[:, db, r * P:(r + 1) * P],
                                    xeT[:, db, j * 512:(j + 1) * 512],
                                    start=(db == 0),
                                    stop=(db == 1),
                                )
                        # psum holds -2*dot; E = (-2*dot + sq_i) + sq_j
                        # drains split between DVE and ACT+gpsimd so neither
                        # engine's in-order stream starves the PE of psum
                        sl = slice(j2 * 1024, (j2 + 1) * 1024)
                        if j2 < 2:
                            nc.vector.scalar_tensor_tensor(
                                s_tile[:, sl], ps.rearrange("p a b -> p (a b)"),
                                sq_all[:, r:r + 1], bcast[:, sl],
                                op0=ADD, op1=ADD)
                        else:
                            nc.scalar.activation(
                                s_tile[:, sl], ps.rearrange("p a b -> p (a b)"),
                                AF.Identity, bias=sq_all[:, r:r + 1], scale=1.0)
                            nc.gpsimd.tensor_add(
                                s_tile[:, sl], s_tile[:, sl], bcast[:, sl])
                    # logits = relu(t * E) in place (== t * relu(E) exactly),
                    # then score = g2 - logits; processed per half-row so the
                    # tail pipelines and the sub is split across DVE/gpsimd
                    h0 = slice(0, N // 2)
                    h1 = slice(N // 2, N)
                    nc.scalar.activation(s_tile[:, h0], s_tile[:, h0],
                                         AF.Relu, scale=t_bc)
                    nc.vector.tensor_sub(q_tile[:, h0], q_tile[:, h0],
                                         s_tile[:, h0])
                    nc.scalar.activation(s_tile[:, h1], s_tile[:, h1],
                                         AF.Relu, scale=t_bc)
                    if r < NBLK_OWN - 1:
                        nc.gpsimd.tensor_sub(q_tile[:, h1], q_tile[:, h1],
                                             s_tile[:, h1])
                    else:
                        # final block: keep the tail chain on the faster DVE
                        nc.vector.tensor_sub(q_tile[:, h1], q_tile[:, h1],
                                             s_tile[:, h1])

                    cv = cvp.tile([P, NSEG * 8], f32, tag="cv")
                    ci = cip.tile([P, NSEG * 8], u16, tag="ci")
                    for s in range(NSEG):
                        nc.vector.max(out=cv[:, s * 8:(s + 1) * 8],
                                      in_=q_tile[:, s * SEG:(s + 1) * SEG])
                    for s in range(NSEG):
                        nc.vector.max_index(ci[:, s * 8:(s + 1) * 8],
                                            cv[:, s * 8:(s + 1) * 8],
                                            q_tile[:, s * SEG:(s + 1) * SEG])
                    nc.sync.dma_start(cv_t[r], cv)
                    nc.sync.dma_start(ci_t[r], ci)

    nc.compile()
    return nc


def _get_program():
    if "nc" not in _CACHE:
        _CACHE["nc"] = _build_program()
    return _CACHE["nc"]


def _run_on_hw(in_maps, trace=False):
    from concourse.bass_utils import run_bass_kernel_spmd
    nc = _get_program()
    if trace:
        try:
            return run_bass_kernel_spmd(
                nc, in_maps, core_ids=list(range(8)), trace=True)
        except Exception as e:  # profiling infra unavailable -> plain run
            print(f"trace run failed ({type(e).__name__}: {e}); retrying untraced")
    return run_bass_kernel_spmd(nc, in_maps, core_ids=list(range(8)), trace=False)


def _host_topk(V, GI):
    """Exact top-16 (desc value, ties -> lower global index) per row."""
    order = np.lexsort((GI, -V.astype(np.float64)), axis=-1)[:, :K]
    val16 = np.take_along_axis(V, order, 1)
    idx16 = np.take_along_axis(GI, order, 1)
    return val16, idx16


def kernel(x, A, W, temperature, q):
    x = np.asarray(x, dtype=np.float32)
    W = np.asarray(W, dtype=np.float32)
    q = np.asarray(q, dtype=np.float32)
    temperature = np.asarray(temperature, dtype=np.float32)

    in_maps = []
    for c in range(8):
        b, h = c // 2, c % 2
        off = h * RPC
        xb = np.roll(x[b], -off, axis=0) if h else x[b]
        qr = q[b, off:off + RPC]
        if h:
            qr = np.roll(qr, -off, axis=1)
        in_maps.append({
            "xb": np.ascontiguousarray(xb),
            "w": W,
            "qr": np.ascontiguousarray(qr),
            "temp": temperature,
        })

    trace = bool(int(os.environ.get("DGM_TRACE", "0")))
    kr = _run_on_hw(in_maps, trace=trace)
    results = kr.results
    if trace and kr.exec_time_ns is not None:
        _CACHE["exec_time_ns"] = kr.exec_time_ns

    # ---- assemble xe ----
    xe = np.empty((B, N, D), dtype=np.float32)
    for c in range(8):
        b, h = c // 2, c % 2
        xe[b, h * RPC:(h + 1) * RPC] = results[c]["xe_out"]

    # ---- candidates -> exact top-16 per row ----
    seg_off = np.repeat(np.arange(NSEG, dtype=np.int32) * SEG, 8)
    t_sc = np.float32(np.exp(np.float32(np.clip(temperature[0], -5.0, 5.0))))

    logprobs = np.empty((B, N, K), dtype=np.float32)
    indices = np.empty((B, N, K), dtype=np.int32)
    n_fallback = 0
    for b in range(B):
        V = np.concatenate(
            [results[2 * b]["cand_v"], results[2 * b + 1]["cand_v"]], axis=0)
        L = np.concatenate(
            [results[2 * b]["cand_i"], results[2 * b + 1]["cand_i"]],
            axis=0).astype(np.int32)
        GI = L + seg_off[None, :]
        GI[RPC:] = (GI[RPC:] + RPC) % N  # undo the key rotation of half 1
        val16, idx16 = _host_topk(V, GI)

        # A segment can hide a relevant element (a 9th element >= the 16th
        # winner, or tied with it) iff its 8th candidate value >= the 16th
        # winner.  Exact condition.
        seg8 = V.reshape(N, NSEG, 8)[:, :, 7]
        bad = (seg8 >= val16[:, K - 1:K]).any(axis=1)
        rows = np.nonzero(bad)[0]
        n_fallback += len(rows)
        if len(rows):
            xeb = xe[b]
            sqb = (xeb * xeb).sum(axis=1, dtype=np.float32)
            dots = (xeb @ xeb[rows].T).T.astype(np.float32)  # [nbad, N]
            E = sqb[rows][:, None] + sqb[None, :] - np.float32(2.0) * dots
            lq = np.maximum(E, np.float32(0.0)) * t_sc \
                - np.log(-np.log(q[b, rows])).astype(np.float32)
            orderr = np.lexsort(
                (np.broadcast_to(np.arange(N), lq.shape), lq), axis=-1)[:, :K]
            idx16[rows] = orderr.astype(np.int32)
            val16[rows] = -np.take_along_axis(lq, orderr, 1)
        logprobs[b] = val16
        indices[b] = idx16
    _CACHE["n_fallback"] = n_fallback

    # ---- edges (reference arithmetic, int32) ----
    rows_arr = np.broadcast_to(
        np.arange(N, dtype=np.int32)[None, :, None], (B, N, K))
    edges = np.stack(
        (indices.reshape(B, -1), rows_arr.reshape(B, -1)), axis=-2)
    offset = (np.arange(B, dtype=np.int32) * N)[:, None, None]
    edges_sparse = np.transpose(edges + offset, (1, 0, 2)).reshape(2, -1)

    return xe, edges_sparse, logprobs


# revision 32
# speedup vs baseline: 2.0094x; 1.0076x over previous
"""Trainium2 Bass kernel for the DGM kNN problem.

Computation (per batch b):
  xe = x @ W                                   [4096, 256]
  D_ij = relu(|xe_i|^2 + |xe_j|^2 - 2 xe_i.xe_j)
  score_ij = ln(-ln(q_ij)) - exp(clip(T,-5,5)) * D_ij     (= -lq)
  top-16 largest score per row i -> (logprobs, indices) -> edges

Sharding: 8 cores = 4 batches x 2 row-halves. Each core receives its
batch's x ROTATED so its own 2048 rows come first (the program is
identical on every core; only data differs). Each core computes the
full 4096x256 embedding (needed for the key side), distances of its
2048 rows against all 4096 keys, and a segmented top-8 per 1024-key
segment (32 candidates/row: values + in-segment indices). The host
merges the 32 candidates into the exact top-16 (jax top_k tie
semantics), with an exact-detected, rare fallback for rows where one
segment hides >8 of the true top-16.

All arithmetic is fp32 and follows the reference's association order
where it matters (relu/scale/subtract orderings are bit-matching; the
dot/sq reductions differ only by summation order, which is inherent to
any reimplementation).
"""

import os
import numpy as np

B, N, D, K = 4, 4096, 256, 16
RPC = N // 2              # rows per core
P = 128                   # partitions
NSEG, SEG = 8, 512        # top-k segments per row
NCH = N // 512            # 512-wide psum chunks per row-block
NBLK_ALL = N // P         # 32 key blocks
NBLK_OWN = RPC // P       # 16 row blocks owned per core

_CACHE = {}


def _build_program():
    import concourse.bacc as bacc
    import concourse.mybir as mybir
    import concourse.tile as tile
    from concourse.masks import make_identity

    f32 = mybir.dt.float32
    u16 = mybir.dt.uint16
    AF = mybir.ActivationFunctionType
    ADD = mybir.AluOpType.add

    nc = bacc.Bacc("TRN2", target_bir_lowering=False, debug=False)

    xb_d = nc.dram_tensor("xb", [N, D], f32, kind="ExternalInput")
    w_d = nc.dram_tensor("w", [D, D], f32, kind="ExternalInput")
    qr_d = nc.dram_tensor("qr", [RPC, N], f32, kind="ExternalInput")
    t_d = nc.dram_tensor("temp", [1], f32, kind="ExternalInput")

    xe_d = nc.dram_tensor("xe_out", [RPC, D], f32, kind="ExternalOutput")
    cv_d = nc.dram_tensor("cand_v", [RPC, NSEG * 8], f32, kind="ExternalOutput")
    ci_d = nc.dram_tensor("cand_i", [RPC, NSEG * 8], u16, kind="ExternalOutput")
    sq_scratch_d = nc.dram_tensor("sq_scratch", [NBLK_ALL, P], f32)

    xb_t = xb_d.ap().rearrange("(t p) d -> t p d", p=P)       # [32,128,256]
    w_t = w_d.ap().rearrange("(b p) e -> b p e", p=P)         # [2,128,256]
    qr_t = qr_d.ap().rearrange("(t p) n -> t p n", p=P)       # [16,128,4096]
    xe_t = xe_d.ap().rearrange("(t p) d -> t p d", p=P)       # [16,128,256]
    cv_t = cv_d.ap().rearrange("(t p) c -> t p c", p=P)
    ci_t = ci_d.ap().rearrange("(t p) c -> t p c", p=P)

    with tile.TileContext(nc) as tc:
        with tc.tile_pool(name="const", bufs=1) as const_pool:
            w_sb = const_pool.tile([P, 2, D], f32)
            xeT = const_pool.tile([P, 2, N], f32)
            xeTm2 = const_pool.tile([P, 2, N], f32)  # -2 * xeT (dist lhsT)
            bcast = const_pool.tile([P, N], f32)
            sq_all = const_pool.tile([P, NBLK_ALL], f32)
            t_bc = const_pool.tile([P, 1], f32)
            ident = const_pool.tile([P, P], f32)
            sq_flat = const_pool.tile([1, N], f32)

            make_identity(nc, ident)
            for db in range(2):
                nc.sync.dma_start(w_sb[:, db, :], w_t[db])

            # temperature -> t_bc = exp(clip(T, -5, 5)) on all partitions
            tt = const_pool.tile([1, 1], f32)
            nc.sync.dma_start(tt, t_d.ap().rearrange("(o a) -> o a", o=1))
            nc.vector.tensor_scalar_max(tt, tt, -5.0)
            nc.vector.tensor_scalar_min(tt, tt, 5.0)
            nc.scalar.activation(tt, tt, AF.Exp)
            nc.gpsimd.partition_broadcast(t_bc, tt)

            # ---------------- setup: xT, xeT(+m2), xe, sq, bcast ----------
            with (
                tc.tile_pool(name="su_in", bufs=6) as su_in,
                tc.tile_pool(name="su_sb", bufs=4) as su_sb,
                tc.tile_pool(name="su_xt", bufs=1) as su_xt,
                tc.tile_pool(name="su_ps", bufs=4, space="PSUM") as su_ps,
                tc.tile_pool(name="su_ps2", bufs=2, space="PSUM") as su_ps2,
                tc.tile_pool(name="su_ps3", bufs=2, space="PSUM") as su_ps3,
            ):
                xT = su_xt.tile([P, 2, N], f32)
                # x -> xT via PE transposes of [128,128] blocks
                for t in range(NBLK_ALL):
                    xin = su_in.tile([P, D], f32, tag="xin")
                    nc.sync.dma_start(xin, xb_t[t])
                    for db in range(2):
                        pst = su_ps.tile([P, P], f32, tag="pst")
                        nc.tensor.transpose(pst, xin[:, db * P:(db + 1) * P], ident)
                        nc.vector.tensor_copy(xT[:, db, t * P:(t + 1) * P], pst)

                # per 512-key chunk: xeT (and -2*xeT) via matmul, then xe
                # row-blocks + sq for the 4 key-blocks covered; every 1024
                # keys completed feeds one bcast chunk of sq_j.
                for ch in range(NCH):
                    for eb in range(2):
                        ps = su_ps2.tile([P, 512], f32, tag="mm")
                        for db in range(2):
                            nc.tensor.matmul(
                                ps,
                                w_sb[:, db, eb * P:(eb + 1) * P],
                                xT[:, db, ch * 512:(ch + 1) * 512],
                                start=(db == 0),
                                stop=(db == 1),
                            )
                        nc.scalar.copy(xeT[:, eb, ch * 512:(ch + 1) * 512], ps)
                        nc.scalar.activation(
                            xeTm2[:, eb, ch * 512:(ch + 1) * 512], ps,
                            AF.Identity, scale=-2.0)
                    for t in range(4 * ch, 4 * ch + 4):
                        xesb = su_sb.tile([P, D], f32, tag="xesb")
                        for eb in range(2):
                            pst = su_ps.tile([P, P], f32, tag="pst")
                            nc.tensor.transpose(
                                pst, xeT[:, eb, t * P:(t + 1) * P], ident)
                            nc.vector.tensor_copy(
                                xesb[:, eb * P:(eb + 1) * P], pst)
                        scr = su_sb.tile([P, D], f32, tag="sqscr")
                        nc.scalar.activation(scr, xesb, AF.Square,
                                             accum_out=sq_all[:, t:t + 1])
                        if t < NBLK_OWN:
                            nc.sync.dma_start(xe_t[t], xesb)
                    if ch % 2 == 1:
                        # keys [c*1024, (c+1)*1024) complete -> bcast chunk c
                        c = ch // 2
                        pssq = su_ps3.tile([8, P], f32, tag="sqt")
                        nc.tensor.transpose(
                            pssq, sq_all[:, c * 8:(c + 1) * 8], ident)
                        sqT = su_sb.tile([8, P], f32, tag="sqT")
                        nc.vector.tensor_copy(sqT, pssq)
                        nc.sync.dma_start(
                            sq_scratch_d.ap()[c * 8:(c + 1) * 8, :], sqT)
                        nc.sync.dma_start(
                            sq_flat[:, c * 1024:(c + 1) * 1024],
                            sq_scratch_d.ap()[c * 8:(c + 1) * 8, :]
                            .rearrange("a b -> (a b)")
                            .rearrange("(o n) -> o n", o=1))
                        nc.gpsimd.partition_broadcast(
                            bcast[:, c * 1024:(c + 1) * 1024],
                            sq_flat[:, c * 1024:(c + 1) * 1024])

            # ---------------- main loop over our 16 row blocks ------------
            with (
                tc.tile_pool(name="qp", bufs=3) as qp,
                tc.tile_pool(name="sp", bufs=3) as sp,
                tc.tile_pool(name="cvp", bufs=2) as cvp,
                tc.tile_pool(name="cip", bufs=2) as cip,
                tc.tile_pool(name="dps", bufs=4, space="PSUM") as dps,
            ):
                for r in range(NBLK_OWN):
                    q_tile = qp.tile([P, N], f32, tag="q")
                    nc.sync.dma_start(q_tile, qr_t[r])
                    # g2 = ln(-ln(q)) in place; half-row ops so psum drains
                    # can interleave into the ACT stream
                    for hh in range(2):
                        hs = slice(hh * (N // 2), (hh + 1) * (N // 2))
                        nc.scalar.activation(q_tile[:, hs], q_tile[:, hs],
                                             AF.Ln)
                        nc.scalar.activation(q_tile[:, hs], q_tile[:, hs],
                                             AF.Ln, scale=-1.0)

                    s_tile = sp.tile([P, N], f32, tag="s")
                    for j2 in range(NCH // 2):
                        ps = dps.tile([P, 2, 512], f32, tag="dot")
                        for jj in range(2):
                            j = j2 * 2 + jj
                            for db in range(2):
                                nc.tensor.matmul(
                                    ps[:, jj, :],
                                    xeTm2[:, db, r * P:(r + 1) * P],
                                    xeT[:, db, j * 512:(j + 1) * 512],
                                    start=(db == 0),
                                    stop=(db == 1),
                                )
                        # psum holds -2*dot; E = (-2*dot + sq_i) + sq_j
                        # drains split between DVE and ACT+gpsimd so neither
                        # engine's in-order stream starves the PE of psum
                        sl = slice(j2 * 1024, (j2 + 1) * 1024)
                        if j2 < 2:
                            nc.vector.scalar_tensor_tensor(
                                s_tile[:, sl], ps.rearrange("p a b -> p (a b)"),
                                sq_all[:, r:r + 1], bcast[:, sl],
                                op0=ADD, op1=ADD)
                        else:
                            nc.scalar.activation(
                                s_tile[:, sl], ps.rearrange("p a b -> p (a b)"),
                                AF.Identity, bias=sq_all[:, r:r + 1], scale=1.0)
                            nc.gpsimd.tensor_add(
                                s_tile[:, sl], s_tile[:, sl], bcast[:, sl])
                    # logits = relu(t * E) in place (== t * relu(E) exactly),
                    # then score = g2 - logits; processed per half-row so the
                    # tail pipelines and the sub is split across DVE/gpsimd
                    h0 = slice(0, N // 2)
                    h1 = slice(N // 2, N)
                    nc.scalar.activation(s_tile[:, h0], s_tile[:, h0],
                                         AF.Relu, scale=t_bc)
                    nc.vector.tensor_sub(q_tile[:, h0], q_tile[:, h0],
                                         s_tile[:, h0])
                    nc.scalar.activation(s_tile[:, h1], s_tile[:, h1],
                                         AF.Relu, scale=t_bc)
                    if r < NBLK_OWN - 1:
                        nc.gpsimd.tensor_sub(q_tile[:, h1], q_tile[:, h1],
                                             s_tile[:, h1])
                    else:
                        # final block: keep the tail chain on the faster DVE
                        nc.vector.tensor_sub(q_tile[:, h1], q_tile[:, h1],
                                             s_tile[:, h1])

                    cv = cvp.tile([P, NSEG * 8], f32, tag="cv")
                    ci = cip.tile([P, NSEG * 8], u16, tag="ci")
                    for s in range(NSEG):
                        nc.vector.max(out=cv[:, s * 8:(s + 1) * 8],
                                      in_=q_tile[:, s * SEG:(s + 1) * SEG])
                    for s in range(NSEG):
                        nc.vector.max_index(ci[:, s * 8:(s + 1) * 8],
                                            cv[:, s * 8:(s + 1) * 8],
                                            q_tile[:, s * SEG:(s + 1) * SEG])
                    nc.sync.dma_start(cv_t[r], cv)
                    nc.sync.dma_start(ci_t[r], ci)

    nc.compile()
    return nc


def _get_program():
    if "nc" not in _CACHE:
        _CACHE["nc"] = _build_program()
    return _CACHE["nc"]


def _run_on_hw(in_maps, trace=False):
    from concourse.bass_utils import run_bass_kernel_spmd
    nc = _get_program()
    if trace:
        try:
            return run_bass_kernel_spmd(
                nc, in_maps, core_ids=list(range(8)), trace=True)
        except Exception as e:  # profiling infra unavailable -> plain run
            print(f"trace run failed ({type(e).__name__}: {e}); retrying untraced")
    return run_bass_kernel_spmd(nc, in_maps, core_ids=list(range(8)), trace=False)


def _host_topk(V, GI):
    """Exact top-16 (desc value, ties -> lower global index) per row."""
    order = np.lexsort((GI, -V.astype(np.float64)), axis=-1)[:, :K]
    val16 = np.take_along_axis(V, order, 1)
    idx16 = np.take_along_axis(GI, order, 1)
    return val16, idx16


def kernel(x, A, W, temperature, q):
    x = np.asarray(x, dtype=np.float32)
    W = np.asarray(W, dtype=np.float32)
    q = np.asarray(q, dtype=np.float32)
    temperature = np.asarray(temperature, dtype=np.float32)

    in_maps = []
    for c in range(8):
        b, h = c // 2, c % 2
        off = h * RPC
        xb = np.roll(x[b], -off, axis=0) if h else x[b]
        qr = q[b, off:off + RPC]
        if h:
            qr = np.roll(qr, -off, axis=1)
        in_maps.append({
            "xb": np.ascontiguousarray(xb),
            "w": W,
            "qr": np.ascontiguousarray(qr),
            "temp": temperature,
        })

    trace = bool(int(os.environ.get("DGM_TRACE", "0")))
    kr = _run_on_hw(in_maps, trace=trace)
    results = kr.results
    if trace and kr.exec_time_ns is not None:
        _CACHE["exec_time_ns"] = kr.exec_time_ns

    # ---- assemble xe ----
    xe = np.empty((B, N, D), dtype=np.float32)
    for c in range(8):
        b, h = c // 2, c % 2
        xe[b, h * RPC:(h + 1) * RPC] = results[c]["xe_out"]

    # ---- candidates -> exact top-16 per row ----
    seg_off = np.repeat(np.arange(NSEG, dtype=np.int32) * SEG, 8)
    t_sc = np.float32(np.exp(np.float32(np.clip(temperature[0], -5.0, 5.0))))

    logprobs = np.empty((B, N, K), dtype=np.float32)
    indices = np.empty((B, N, K), dtype=np.int32)
    n_fallback = 0
    for b in range(B):
        V = np.concatenate(
            [results[2 * b]["cand_v"], results[2 * b + 1]["cand_v"]], axis=0)
        L = np.concatenate(
            [results[2 * b]["cand_i"], results[2 * b + 1]["cand_i"]],
            axis=0).astype(np.int32)
        GI = L + seg_off[None, :]
        GI[RPC:] = (GI[RPC:] + RPC) % N  # undo the key rotation of half 1
        val16, idx16 = _host_topk(V, GI)

        # A segment can hide a relevant element (a 9th element >= the 16th
        # winner, or tied with it) iff its 8th candidate value >= the 16th
        # winner.  Exact condition.
        seg8 = V.reshape(N, NSEG, 8)[:, :, 7]
        bad = (seg8 >= val16[:, K - 1:K]).any(axis=1)
        rows = np.nonzero(bad)[0]
        n_fallback += len(rows)
        if len(rows):
            xeb = xe[b]
            sqb = (xeb * xeb).sum(axis=1, dtype=np.float32)
            dots = (xeb @ xeb[rows].T).T.astype(np.float32)  # [nbad, N]
            E = sqb[rows][:, None] + sqb[None, :] - np.float32(2.0) * dots
            lq = np.maximum(E, np.float32(0.0)) * t_sc \
                - np.log(-np.log(q[b, rows])).astype(np.float32)
            orderr = np.lexsort(
                (np.broadcast_to(np.arange(N), lq.shape), lq), axis=-1)[:, :K]
            idx16[rows] = orderr.astype(np.int32)
            val16[rows] = -np.take_along_axis(lq, orderr, 1)
        logprobs[b] = val16
        indices[b] = idx16
    _CACHE["n_fallback"] = n_fallback

    # ---- edges (reference arithmetic, int32) ----
    rows_arr = np.broadcast_to(
        np.arange(N, dtype=np.int32)[None, :, None], (B, N, K))
    edges = np.stack(
        (indices.reshape(B, -1), rows_arr.reshape(B, -1)), axis=-2)
    offset = (np.arange(B, dtype=np.int32) * N)[:, None, None]
    edges_sparse = np.transpose(edges + offset, (1, 0, 2)).reshape(2, -1)

    return xe, edges_sparse, logprobs
